# revision 28
# baseline (speedup 1.0000x reference)
"""Trainium2 Bass kernel for the histogram_binning problem.

Math (per batch sample b):
  h = x[b] viewed as [C, N]  (C=2208 channels, N=196 positions)
  z[n, k] = sum_c h[c, n] * W[k, c] + bias[k]          (K=200 classes)
  max_val[n]  = max_k softmax(z[n,:]) = 1 / sum_k exp(z[n,k] - zmax[n])
  max_ids[n]  = argmax_k z[n, :]
  norm = max_val / ||max_val||_2
  p_r[k] = (sum_{n: ids[n]=k} max_val[n]) / L1   (L2 scale cancels under L1 norm)
  out[c, n] = x[c, n] * (1 + norm[n])

Distribution: pure data parallel, batch 64 -> 8 cores x 8 samples.

Implementation notes:
 - x is host-padded [C=2208] -> [CP=2304 = 18*128] rows; flat row 2208 is all
   ones and W row 2208 is fc_b, folding the bias add into the contraction.
 - Channel c maps to (partition p, chunk j) = (c // 18, c % 18) so the x DMA
   per sample is contiguous (14KB per partition). The weights are
   host-permuted the same way, so matmul j contracts the stride-18 channel
   subset {18p + j}; summed over j this is the full C sum.
 - The z matmul runs in fp16 hi/lo split precision: x*64 and W*1024 are each
   split into fp16 (hi, lo) pairs on the host; z_s = 65536*z is accumulated
   from 3 fp16 matmuls (hi*hi + hi*lo + lo*hi) at 1 cycle/row with fast
   weight load, vs fp32's 4 cycles/row + serial LDWEIGHTS. The scaling keeps
   every lo part in fp16 normal range, so the representation error is
   ~2^-22 relative; the exp stage folds the 2^-16 unscale into its scale
   argument, and the argmax mask is scale-invariant.
 - argmax one-hot is computed as (z_s - zmax_s == 0) elementwise on PSUM.
 - The scatter-add histogram matmul mv^T @ [mask | mv | ones] also yields
   sum(mv^2) (for the L2 norm) and sum(mv) (= L1 of the histogram) for free.
 - 1/sqrt(ssq) is computed as exp(-0.5*ln(ssq)): ln/exp/copy/square live in
   one ACT table set, avoiding the ~2.7us-per-swap sqrt table thrash.
"""

import numpy as np

import concourse.bass as bass
import concourse.bacc as bacc
import concourse.mybir as mybir
import concourse.tile as tile
from concourse.bass_utils import run_bass_kernel_spmd
from concourse.masks import make_identity

F32 = mybir.dt.float32
F16 = mybir.dt.float16
XS = 64.0           # host scale for x
WS = 1024.0         # host scale for W
ZI = 1.0 / (XS * WS)  # 2^-16, exact

B = 64
C = 2208
CP = 2304            # padded channel dim: 18 * 128 (flat row 2208 = bias ones)
H = W = 14
N = H * W            # 196
K = 200
KE = K + 2           # mask cols: [one-hot(200) | mv | ones]
NCORES = 8
BPC = B // NCORES    # 8 samples per core
CT = CP // 128       # 18 contraction chunks
NT = ((0, 128), (128, 68))   # (offset, size) tiles of N=196
DVE_J = 16           # final multiply: chunks [0, DVE_J) on DVE, rest on gpsimd
XP = ((0, 2), (2, 6), (6, 12), (12, 18))     # x/w DMA piece boundaries
MP = ((0, 4), (4, 8), (8, 12), (12, 16))     # DVE multiply piece boundaries


def _pin_act_table_set():
    """Steer Bacc's act-table-load pass to one set that covers every
    activation we use (exp, ln, copy, square, identity), so the kernel does a
    single ACT_TABLE_LOAD instead of thrashing ~2.7us swaps between the
    default per-function first-match sets. Set ids/order are preserved; we
    only hide functions from the other sets."""
    import concourse.hw_specs as hw_specs

    if getattr(hw_specs.get_activation_tables, "_pinned", False):
        return
    orig = hw_specs.get_activation_tables

    @hw_specs.functools.cache
    def pinned(module_arch):
        tables = dict(orig(module_arch))
        keep = "natural_log_exp_and_others"
        if keep in tables:
            ours = {
                mybir.ActivationFunctionType.Exp,
                mybir.ActivationFunctionType.Ln,
                mybir.ActivationFunctionType.Copy,
                mybir.ActivationFunctionType.Identity,
                mybir.ActivationFunctionType.Square,
            }
            if ours <= tables[keep]:
                tables = {
                    name: (fns if name == keep else fns - ours)
                    for name, fns in tables.items()
                }
        return tables

    pinned._pinned = True
    hw_specs.get_activation_tables = pinned
    import concourse.bacc as _bacc_mod

    _bacc_mod.get_activation_tables = pinned


def _build_nc() -> bass.Bass:
    _pin_act_table_set()
    nc = bacc.Bacc(None, target_bir_lowering=False, debug=False)
    xh_d = nc.declare_dram_parameter("xh", [BPC, 128, CT * N], F16, isOutput=False)
    xl_d = nc.declare_dram_parameter("xl", [BPC, 128, CT * N], F16, isOutput=False)
    wh_d = nc.declare_dram_parameter("wh", [128, CT * K], F16, isOutput=False)
    wl_d = nc.declare_dram_parameter("wl", [128, CT * K], F16, isOutput=False)
    out_d = nc.declare_dram_parameter("yo", [BPC, 128, CT * N], F32, isOutput=True)
    pr_d = nc.declare_dram_parameter("pr", [BPC, K], F32, isOutput=True)

    with tile.TileContext(nc) as tc:
        with (
            tc.tile_pool(name="consts", bufs=1) as consts,
            tc.tile_pool(name="xpool", bufs=1) as xpool,
            tc.tile_pool(name="xhl", bufs=3) as xhl,
            tc.tile_pool(name="maskp", bufs=4) as maskp,
            tc.tile_pool(name="escr", bufs=3) as escr,
            tc.tile_pool(name="stats", bufs=6) as stats,
            tc.tile_pool(name="brow", bufs=3) as brow,
            tc.tile_pool(name="tinyp", bufs=3) as tinyp,
            tc.tile_pool(name="bcsb", bufs=3) as bcsb,
            tc.tile_pool(name="psz", bufs=4, space="PSUM") as psz_pool,
            tc.tile_pool(name="pspr", bufs=2, space="PSUM") as pspr_pool,
            tc.tile_pool(name="pstr", bufs=1, space="PSUM") as pstr_pool,
            tc.tile_pool(name="psbc", bufs=1, space="PSUM") as psbc_pool,
        ):
            # --- constants ---
            wh_sb = consts.tile([128, CT, K], F16)
            wl_sb = consts.tile([128, CT, K], F16)
            wh_v = wh_d[:, :].rearrange("p (t k) -> p t k", t=CT)
            wl_v = wl_d[:, :].rearrange("p (t k) -> p t k", t=CT)
            for wa, wb_ in ((0, 2), (2, 18)):
                nc.sync.dma_start(out=wh_sb[:, wa:wb_, :], in_=wh_v[:, wa:wb_, :])
                nc.sync.dma_start(out=wl_sb[:, wa:wb_, :], in_=wl_v[:, wa:wb_, :])
            ident = consts.tile([128, 128], F32)
            make_identity(nc, ident)
            ones_row = consts.tile([1, 128], F32)
            nc.gpsimd.memset(ones_row, 1.0)

            for b in range(BPC):
                # --- load x[b] in 4 contiguous pieces ---
                x_b = xpool.tile([128, CT, N], F32, tag=f"x{b}")
                xh_b = xhl.tile([128, CT, N], F16, tag="xh")
                xl_b = xhl.tile([128, CT, N], F16, tag="xl")
                xh_v = xh_d[b].rearrange("p (t n) -> p t n", t=CT)
                xl_v = xl_d[b].rearrange("p (t n) -> p t n", t=CT)
                for xa, xb_ in ((0, 2), (2, 9), (9, 18)):
                    nc.sync.dma_start(out=xh_b[:, xa:xb_, :], in_=xh_v[:, xa:xb_, :])
                    nc.sync.dma_start(out=xl_b[:, xa:xb_, :], in_=xl_v[:, xa:xb_, :])
                # reconstruct 64*x = hi + lo in fp32 for the output scaling;
                # runs on DVE/gpsimd concurrently with this sample's matmuls
                for ja, jb, eng in ((0, 8, nc.vector), (8, 16, nc.vector),
                                    (16, CT, nc.gpsimd)):
                    eng.tensor_tensor(
                        x_b[:, ja:jb, :],
                        xh_b[:, ja:jb, :],
                        xl_b[:, ja:jb, :],
                        op=mybir.AluOpType.add,
                    )

                pspr = pspr_pool.tile([1, KE], F32, tag="pr")
                pstr = pstr_pool.tile([1, N], F32, tag="tr")

                for i, (noff, nsz) in enumerate(NT):
                    # z_s = 65536 * (x^T W + bias): 3 fp16 matmuls per chunk
                    psz = psz_pool.tile([128, K], F32, tag="z")
                    for t in range(CT):
                        nc.tensor.matmul(
                            psz[:nsz, :],
                            lhsT=xh_b[:, t, noff : noff + nsz],
                            rhs=wh_sb[:, t, :],
                            start=(t == 0),
                            stop=False,
                        )
                        nc.tensor.matmul(
                            psz[:nsz, :],
                            lhsT=xh_b[:, t, noff : noff + nsz],
                            rhs=wl_sb[:, t, :],
                            start=False,
                            stop=False,
                        )
                        nc.tensor.matmul(
                            psz[:nsz, :],
                            lhsT=xl_b[:, t, noff : noff + nsz],
                            rhs=wh_sb[:, t, :],
                            start=False,
                            stop=(t == CT - 1),
                        )
                    # negmax[n] = -max_k z
                    negmax = stats.tile([128, 1], F32, tag="negmax")
                    nc.vector.tensor_reduce(
                        out=negmax[:nsz],
                        in_=psz[:nsz, :],
                        axis=mybir.AxisListType.X,
                        op=mybir.AluOpType.max,
                        negate=True,
                    )
                    # sumexp[n] = sum_k exp((z_s - zmax_s) * 2^-16)
                    negmax_sc = stats.tile([128, 1], F32, tag="negmaxsc")
                    nc.scalar.mul(negmax_sc[:nsz], negmax[:nsz], ZI)
                    e_scr = escr.tile([128, K], F32, tag="escr")
                    sumexp = stats.tile([128, 1], F32, tag="sumexp")
                    nc.scalar.activation(
                        out=e_scr[:nsz],
                        in_=psz[:nsz, :],
                        func=mybir.ActivationFunctionType.Exp,
                        bias=negmax_sc[:nsz],
                        scale=ZI,
                        accum_out=sumexp[:nsz],
                    )
                    # mask = [one-hot argmax (z + negmax == 0) | mv | ones]
                    mask = maskp.tile([128, KE], F32, tag="mask")
                    nc.vector.tensor_scalar(
                        mask[:nsz, :K],
                        psz[:nsz, :],
                        negmax[:nsz],
                        0.0,
                        op0=mybir.AluOpType.add,
                        op1=mybir.AluOpType.is_equal,
                    )
                    # max_val[n] = 1 / sumexp  -> mask col 200
                    nc.vector.reciprocal(mask[:nsz, K : K + 1], sumexp[:nsz])
                    nc.gpsimd.memset(mask[:nsz, K + 1 : K + 2], 1.0)
                    # p_r_raw | ssq | L1  =  mv^T @ [mask | mv | ones]
                    nc.tensor.matmul(
                        pspr[:, :],
                        lhsT=mask[:nsz, K : K + 1],
                        rhs=mask[:nsz, :],
                        start=(i == 0),
                        stop=(i == 1),
                    )
                    # transpose max_val column into a row [1, N]
                    nc.tensor.transpose(
                        pstr[0:1, noff : noff + nsz],
                        mask[:nsz, K : K + 1],
                        ident[:nsz, :nsz],
                    )

                # --- per-sample tail: normalize + broadcast + scale ---
                mvrow = brow.tile([1, N], F32, tag="mvrow")
                nc.scalar.copy(mvrow, pstr[0:1, :])
                # rl2 = 1/sqrt(ssq) = exp(-0.5 * ln(ssq)); ssq = pspr[0, 200]
                lssq = tinyp.tile([1, 1], F32, tag="lssq")
                nc.scalar.activation(
                    out=lssq,
                    in_=pspr[0:1, K : K + 1],
                    func=mybir.ActivationFunctionType.Ln,
                )
                rl2 = tinyp.tile([1, 1], F32, tag="rl2")
                nc.scalar.activation(
                    out=rl2,
                    in_=lssq,
                    func=mybir.ActivationFunctionType.Exp,
                    scale=-0.5,
                )
                # x_b holds 64*x, so scale by (1 + norm)/64:
                # n1row = (max_val_row * rl2 + 1) / 64; fold /64 into a
                # rescaled rl2 and constant 1/64.
                rl2s = tinyp.tile([1, 1], F32, tag="rl2s")
                nc.vector.tensor_scalar_mul(rl2s, rl2, 1.0 / XS)
                n1row = brow.tile([1, N], F32, tag="n1row")
                nc.vector.tensor_scalar(
                    n1row,
                    mvrow,
                    rl2s,
                    1.0 / XS,
                    op0=mybir.AluOpType.mult,
                    op1=mybir.AluOpType.add,
                )
                # broadcast row to all 128 partitions: ones[1,128]^T @ n1row[1,N]
                psbc = psbc_pool.tile([128, N], F32, tag="bc")
                nc.tensor.matmul(psbc, lhsT=ones_row, rhs=n1row)
                bc_sb = bcsb.tile([128, N], F32, tag="bcsb")
                nc.vector.tensor_copy(bc_sb, psbc)

                # out = x * (1 + norm), in place; DVE pieces pipelined with the
                # out-DMA, last 2 chunks on gpsimd in parallel
                out_v = out_d[b].rearrange("p (t n) -> p t n", t=CT)
                for ja, jb in MP:
                    nc.vector.tensor_tensor(
                        x_b[:, ja:jb, :],
                        x_b[:, ja:jb, :],
                        bc_sb[:, None, :].to_broadcast((128, jb - ja, N)),
                        op=mybir.AluOpType.mult,
                    )
                    nc.sync.dma_start(out=out_v[:, ja:jb, :], in_=x_b[:, ja:jb, :])
                nc.gpsimd.tensor_tensor(
                    x_b[:, DVE_J:, :],
                    x_b[:, DVE_J:, :],
                    bc_sb[:, None, :].to_broadcast((128, CT - DVE_J, N)),
                    op=mybir.AluOpType.mult,
                )
                nc.sync.dma_start(out=out_v[:, DVE_J:, :], in_=x_b[:, DVE_J:, :])

                # --- p_r row: L1 normalize (L1 = pspr[0, 201]) and store ---
                rl1 = tinyp.tile([1, 1], F32, tag="rl1")
                nc.vector.reciprocal(rl1, pspr[0:1, K + 1 : K + 2])
                pr_sb = brow.tile([1, K], F32, tag="prsb")
                nc.vector.tensor_scalar_mul(pr_sb, pspr[0:1, :K], rl1)
                nc.sync.dma_start(out=pr_d[b : b + 1, :], in_=pr_sb)

    nc.compile()
    return nc


_NC = None


def _get_nc():
    global _NC
    if _NC is None:
        _NC = _build_nc()
    return _NC


def _pack_weights(fc_w: np.ndarray, fc_b: np.ndarray):
    """fp16 (hi, lo) pair of 1024*W^T padded to CP rows (row 2208 = fc_b),
    permuted so partition p chunk j holds channel c = 18p + j."""
    wp = np.zeros((CP, K), dtype=np.float32)
    wp[:C] = fc_w.astype(np.float32, copy=False).T
    wp[C] = fc_b.astype(np.float32, copy=False)
    wp *= WS
    wh = wp.astype(np.float16)
    wl = (wp - wh.astype(np.float32)).astype(np.float16)
    return (
        np.ascontiguousarray(wh.reshape(128, CT * K)),
        np.ascontiguousarray(wl.reshape(128, CT * K)),
    )


def _pad_x(x: np.ndarray):
    """fp16 (hi, lo) pair of 64*x padded to CP rows (row 2208 = 64 ones)."""
    xp = np.zeros((B, CP, N), dtype=np.float32)
    xp[:, :C] = x.reshape(B, C, N)
    xp[:, C] = 1.0
    xp = xp.reshape(B, 128, CT * N)
    xs64 = xp * np.float32(XS)
    xh = xs64.astype(np.float16)
    xl = (xs64 - xh.astype(np.float32)).astype(np.float16)
    return xh, xl


def _run(x, fc_w, fc_b, flag, trace=False, trace_cores=None):
    x = np.asarray(x, dtype=np.float32)
    xh, xl = _pad_x(x)
    wh, wl = _pack_weights(np.asarray(fc_w), np.asarray(fc_b))
    in_maps = [
        {
            "xh": np.ascontiguousarray(xh[i * BPC : (i + 1) * BPC]),
            "xl": np.ascontiguousarray(xl[i * BPC : (i + 1) * BPC]),
            "wh": wh,
            "wl": wl,
        }
        for i in range(NCORES)
    ]
    nc = _get_nc()
    res = run_bass_kernel_spmd(
        nc,
        in_maps,
        core_ids=list(range(NCORES)),
        trace=trace,
        **({"trace_cores": trace_cores} if trace_cores else {}),
    )
    out = np.concatenate(
        [r["yo"].reshape(BPC, CP, N)[:, :C] for r in res.results], axis=0
    )
    out = out.reshape(B, C, H, W)
    p_r = np.concatenate([r["pr"] for r in res.results], axis=0)
    if not int(np.asarray(flag)):
        p_r = np.zeros_like(p_r)
    return (out, p_r), res


def kernel(x, fc_w, fc_b, flag):
    (out, p_r), _ = _run(x, fc_w, fc_b, flag)
    return out, p_r


# revision 29
# speedup vs baseline: 1.0368x; 1.0368x over previous
"""Trainium2 Bass kernel for the histogram_binning problem.

Math (per batch sample b):
  h = x[b] viewed as [C, N]  (C=2208 channels, N=196 positions)
  z[n, k] = sum_c h[c, n] * W[k, c] + bias[k]          (K=200 classes)
  max_val[n]  = max_k softmax(z[n,:]) = 1 / sum_k exp(z[n,k] - zmax[n])
  max_ids[n]  = argmax_k z[n, :]
  norm = max_val / ||max_val||_2
  p_r[k] = (sum_{n: ids[n]=k} max_val[n]) / L1   (L2 scale cancels under L1 norm)
  out[c, n] = x[c, n] * (1 + norm[n])

Distribution: pure data parallel, batch 64 -> 8 cores x 8 samples.

Implementation notes:
 - x is host-padded [C=2208] -> [CP=2304 = 18*128] rows; flat row 2208 is all
   ones and W row 2208 is fc_b, folding the bias add into the contraction.
 - Channel c maps to (partition p, chunk j) = (c // 18, c % 18) so the x DMA
   per sample is contiguous (14KB per partition). The weights are
   host-permuted the same way, so matmul j contracts the stride-18 channel
   subset {18p + j}; summed over j this is the full C sum.
 - The z matmul runs in fp16 hi/lo split precision: x*64 and W*1024 are each
   split into fp16 (hi, lo) pairs on the host; z_s = 65536*z is accumulated
   from 3 fp16 matmuls (hi*hi + hi*lo + lo*hi) at 1 cycle/row with fast
   weight load, vs fp32's 4 cycles/row + serial LDWEIGHTS. The scaling keeps
   every lo part in fp16 normal range, so the representation error is
   ~2^-22 relative; the exp stage folds the 2^-16 unscale into its scale
   argument, and the argmax mask is scale-invariant.
 - argmax one-hot is computed as (z_s - zmax_s == 0) elementwise on PSUM.
 - The scatter-add histogram matmul mv^T @ [mask | mv | ones] also yields
   sum(mv^2) (for the L2 norm) and sum(mv) (= L1 of the histogram) for free.
 - 1/sqrt(ssq) is computed as exp(-0.5*ln(ssq)): ln/exp/copy/square live in
   one ACT table set, avoiding the ~2.7us-per-swap sqrt table thrash.
"""

import numpy as np

import concourse.bass as bass
import concourse.bacc as bacc
import concourse.mybir as mybir
import concourse.tile as tile
from concourse.bass_utils import run_bass_kernel_spmd
from concourse.masks import make_identity

F32 = mybir.dt.float32
F16 = mybir.dt.float16
XS = 64.0           # host scale for x
WS = 1024.0         # host scale for W
ZI = 1.0 / (XS * WS)  # 2^-16, exact

B = 64
C = 2208
CP = 2304            # padded channel dim: 18 * 128 (flat row 2208 = bias ones)
H = W = 14
N = H * W            # 196
K = 200
KE = K + 2           # mask cols: [one-hot(200) | mv | ones]
NCORES = 8
BPC = B // NCORES    # 8 samples per core
CT = CP // 128       # 18 contraction chunks
NT = ((0, 128), (128, 68))   # (offset, size) tiles of N=196
DVE_J = 16           # final multiply: chunks [0, DVE_J) on DVE, rest on gpsimd
XP = ((0, 2), (2, 6), (6, 12), (12, 18))     # x/w DMA piece boundaries
MP = ((0, 4), (4, 8), (8, 12), (12, 16))     # DVE multiply piece boundaries


def _pin_act_table_set():
    """Steer Bacc's act-table-load pass to one set that covers every
    activation we use (exp, ln, copy, square, identity), so the kernel does a
    single ACT_TABLE_LOAD instead of thrashing ~2.7us swaps between the
    default per-function first-match sets. Set ids/order are preserved; we
    only hide functions from the other sets."""
    import concourse.hw_specs as hw_specs

    if getattr(hw_specs.get_activation_tables, "_pinned", False):
        return
    orig = hw_specs.get_activation_tables

    @hw_specs.functools.cache
    def pinned(module_arch):
        tables = dict(orig(module_arch))
        keep = "natural_log_exp_and_others"
        if keep in tables:
            ours = {
                mybir.ActivationFunctionType.Exp,
                mybir.ActivationFunctionType.Ln,
                mybir.ActivationFunctionType.Copy,
                mybir.ActivationFunctionType.Identity,
                mybir.ActivationFunctionType.Square,
            }
            if ours <= tables[keep]:
                tables = {
                    name: (fns if name == keep else fns - ours)
                    for name, fns in tables.items()
                }
        return tables

    pinned._pinned = True
    hw_specs.get_activation_tables = pinned
    import concourse.bacc as _bacc_mod

    _bacc_mod.get_activation_tables = pinned


def _build_nc() -> bass.Bass:
    _pin_act_table_set()
    nc = bacc.Bacc(None, target_bir_lowering=False, debug=False)
    xh_d = nc.declare_dram_parameter("xh", [BPC, 128, CT * N], F16, isOutput=False)
    xl_d = nc.declare_dram_parameter("xl", [BPC, 128, CT * N], F16, isOutput=False)
    wh_d = nc.declare_dram_parameter("wh", [128, CT * K], F16, isOutput=False)
    wl_d = nc.declare_dram_parameter("wl", [128, CT * K], F16, isOutput=False)
    out_d = nc.declare_dram_parameter("yo", [BPC, 128, CT * N], F32, isOutput=True)
    pr_d = nc.declare_dram_parameter("pr", [BPC, K], F32, isOutput=True)

    with tile.TileContext(nc) as tc:
        with (
            tc.tile_pool(name="consts", bufs=1) as consts,
            tc.tile_pool(name="xpool", bufs=1) as xpool,
            tc.tile_pool(name="xhl", bufs=3) as xhl,
            tc.tile_pool(name="maskp", bufs=4) as maskp,
            tc.tile_pool(name="escr", bufs=3) as escr,
            tc.tile_pool(name="stats", bufs=6) as stats,
            tc.tile_pool(name="brow", bufs=3) as brow,
            tc.tile_pool(name="tinyp", bufs=3) as tinyp,
            tc.tile_pool(name="bcsb", bufs=3) as bcsb,
            tc.tile_pool(name="psz", bufs=4, space="PSUM") as psz_pool,
            tc.tile_pool(name="pspr", bufs=2, space="PSUM") as pspr_pool,
            tc.tile_pool(name="pstr", bufs=1, space="PSUM") as pstr_pool,
            tc.tile_pool(name="psbc", bufs=1, space="PSUM") as psbc_pool,
        ):
            # --- constants ---
            wh_sb = consts.tile([128, CT, K], F16)
            wl_sb = consts.tile([128, CT, K], F16)
            wh_v = wh_d[:, :].rearrange("p (t k) -> p t k", t=CT)
            wl_v = wl_d[:, :].rearrange("p (t k) -> p t k", t=CT)
            for wa, wb_ in ((0, 2), (2, 18)):
                nc.sync.dma_start(out=wh_sb[:, wa:wb_, :], in_=wh_v[:, wa:wb_, :])
                nc.sync.dma_start(out=wl_sb[:, wa:wb_, :], in_=wl_v[:, wa:wb_, :])
            ident = consts.tile([128, 128], F32)
            make_identity(nc, ident)
            ones_row = consts.tile([1, 128], F32)
            nc.gpsimd.memset(ones_row, 1.0)

            for b in range(BPC):
                # --- load x[b] in 4 contiguous pieces ---
                x_b = xpool.tile([128, CT, N], F32, tag=f"x{b}")
                xh_b = xhl.tile([128, CT, N], F16, tag="xh")
                xl_b = xhl.tile([128, CT, N], F16, tag="xl")
                xh_v = xh_d[b].rearrange("p (t n) -> p t n", t=CT)
                xl_v = xl_d[b].rearrange("p (t n) -> p t n", t=CT)
                for xa, xb_ in ((0, 2), (2, 9), (9, 18)):
                    nc.sync.dma_start(out=xh_b[:, xa:xb_, :], in_=xh_v[:, xa:xb_, :])
                    nc.sync.dma_start(out=xl_b[:, xa:xb_, :], in_=xl_v[:, xa:xb_, :])
                # reconstruct 64*x = hi + lo in fp32 for the output scaling;
                # runs on DVE/gpsimd concurrently with this sample's matmuls
                for ja, jb, eng in ((0, 8, nc.vector), (8, 16, nc.vector),
                                    (16, CT, nc.gpsimd)):
                    eng.tensor_tensor(
                        x_b[:, ja:jb, :],
                        xh_b[:, ja:jb, :],
                        xl_b[:, ja:jb, :],
                        op=mybir.AluOpType.add,
                    )

                pspr = pspr_pool.tile([1, KE], F32, tag="pr")
                pstr = pstr_pool.tile([1, N], F32, tag="tr")

                for i, (noff, nsz) in enumerate(NT):
                    # z_s = 65536 * (x^T W + bias): 3 fp16 matmuls per chunk
                    psz = psz_pool.tile([128, K], F32, tag="z")
                    for t in range(CT):
                        nc.tensor.matmul(
                            psz[:nsz, :],
                            lhsT=xh_b[:, t, noff : noff + nsz],
                            rhs=wh_sb[:, t, :],
                            start=(t == 0),
                            stop=False,
                        )
                        nc.tensor.matmul(
                            psz[:nsz, :],
                            lhsT=xh_b[:, t, noff : noff + nsz],
                            rhs=wl_sb[:, t, :],
                            start=False,
                            stop=False,
                        )
                        nc.tensor.matmul(
                            psz[:nsz, :],
                            lhsT=xl_b[:, t, noff : noff + nsz],
                            rhs=wh_sb[:, t, :],
                            start=False,
                            stop=(t == CT - 1),
                        )
                    # negmax[n] = -max_k z
                    negmax = stats.tile([128, 1], F32, tag="negmax")
                    nc.vector.tensor_reduce(
                        out=negmax[:nsz],
                        in_=psz[:nsz, :],
                        axis=mybir.AxisListType.X,
                        op=mybir.AluOpType.max,
                        negate=True,
                    )
                    # sumexp[n] = sum_k exp((z_s - zmax_s) * 2^-16)
                    negmax_sc = stats.tile([128, 1], F32, tag="negmaxsc")
                    nc.vector.tensor_scalar_mul(
                        negmax_sc[:nsz], negmax[:nsz], ZI
                    )
                    e_scr = escr.tile([128, K], F32, tag="escr")
                    sumexp = stats.tile([128, 1], F32, tag="sumexp")
                    nc.scalar.activation(
                        out=e_scr[:nsz],
                        in_=psz[:nsz, :],
                        func=mybir.ActivationFunctionType.Exp,
                        bias=negmax_sc[:nsz],
                        scale=ZI,
                        accum_out=sumexp[:nsz],
                    )
                    # mask = [one-hot argmax (z + negmax == 0) | mv | ones]
                    mask = maskp.tile([128, KE], F32, tag="mask")
                    nc.vector.tensor_scalar(
                        mask[:nsz, :K],
                        psz[:nsz, :],
                        negmax[:nsz],
                        0.0,
                        op0=mybir.AluOpType.add,
                        op1=mybir.AluOpType.is_equal,
                    )
                    # max_val[n] = 1 / sumexp  -> mask col 200
                    nc.vector.reciprocal(mask[:nsz, K : K + 1], sumexp[:nsz])
                    nc.gpsimd.memset(mask[:nsz, K + 1 : K + 2], 1.0)
                    # p_r_raw | ssq | L1  =  mv^T @ [mask | mv | ones]
                    nc.tensor.matmul(
                        pspr[:, :],
                        lhsT=mask[:nsz, K : K + 1],
                        rhs=mask[:nsz, :],
                        start=(i == 0),
                        stop=(i == 1),
                    )
                    # transpose max_val column into a row [1, N]
                    nc.tensor.transpose(
                        pstr[0:1, noff : noff + nsz],
                        mask[:nsz, K : K + 1],
                        ident[:nsz, :nsz],
                    )

                # --- per-sample tail: normalize + broadcast + scale ---
                mvrow = brow.tile([1, N], F32, tag="mvrow")
                nc.scalar.copy(mvrow, pstr[0:1, :])
                # rl2 = 1/sqrt(ssq) = exp(-0.5 * ln(ssq)); ssq = pspr[0, 200]
                lssq = tinyp.tile([1, 1], F32, tag="lssq")
                nc.scalar.activation(
                    out=lssq,
                    in_=pspr[0:1, K : K + 1],
                    func=mybir.ActivationFunctionType.Ln,
                )
                rl2 = tinyp.tile([1, 1], F32, tag="rl2")
                nc.scalar.activation(
                    out=rl2,
                    in_=lssq,
                    func=mybir.ActivationFunctionType.Exp,
                    scale=-0.5,
                )
                # x_b holds 64*x, so scale by (1 + norm)/64:
                # n1row = (max_val_row * rl2 + 1) / 64; fold /64 into a
                # rescaled rl2 and constant 1/64.
                rl2s = tinyp.tile([1, 1], F32, tag="rl2s")
                nc.vector.tensor_scalar_mul(rl2s, rl2, 1.0 / XS)
                n1row = brow.tile([1, N], F32, tag="n1row")
                nc.vector.tensor_scalar(
                    n1row,
                    mvrow,
                    rl2s,
                    1.0 / XS,
                    op0=mybir.AluOpType.mult,
                    op1=mybir.AluOpType.add,
                )
                # broadcast row to all 128 partitions: ones[1,128]^T @ n1row[1,N]
                psbc = psbc_pool.tile([128, N], F32, tag="bc")
                nc.tensor.matmul(psbc, lhsT=ones_row, rhs=n1row)
                bc_sb = bcsb.tile([128, N], F32, tag="bcsb")
                nc.vector.tensor_copy(bc_sb, psbc)

                # out = x * (1 + norm), in place; DVE pieces pipelined with the
                # out-DMA, last 2 chunks on gpsimd in parallel
                out_v = out_d[b].rearrange("p (t n) -> p t n", t=CT)
                for ja, jb in MP:
                    nc.vector.tensor_tensor(
                        x_b[:, ja:jb, :],
                        x_b[:, ja:jb, :],
                        bc_sb[:, None, :].to_broadcast((128, jb - ja, N)),
                        op=mybir.AluOpType.mult,
                    )
                    nc.sync.dma_start(out=out_v[:, ja:jb, :], in_=x_b[:, ja:jb, :])
                nc.gpsimd.tensor_tensor(
                    x_b[:, DVE_J:, :],
                    x_b[:, DVE_J:, :],
                    bc_sb[:, None, :].to_broadcast((128, CT - DVE_J, N)),
                    op=mybir.AluOpType.mult,
                )
                nc.sync.dma_start(out=out_v[:, DVE_J:, :], in_=x_b[:, DVE_J:, :])

                # --- p_r row: L1 normalize (L1 = pspr[0, 201]) and store ---
                rl1 = tinyp.tile([1, 1], F32, tag="rl1")
                nc.vector.reciprocal(rl1, pspr[0:1, K + 1 : K + 2])
                pr_sb = brow.tile([1, K], F32, tag="prsb")
                nc.vector.tensor_scalar_mul(pr_sb, pspr[0:1, :K], rl1)
                nc.sync.dma_start(out=pr_d[b : b + 1, :], in_=pr_sb)

    nc.compile()
    return nc


_NC = None


def _get_nc():
    global _NC
    if _NC is None:
        _NC = _build_nc()
    return _NC


def _pack_weights(fc_w: np.ndarray, fc_b: np.ndarray):
    """fp16 (hi, lo) pair of 1024*W^T padded to CP rows (row 2208 = fc_b),
    permuted so partition p chunk j holds channel c = 18p + j."""
    wp = np.zeros((CP, K), dtype=np.float32)
    wp[:C] = fc_w.astype(np.float32, copy=False).T
    wp[C] = fc_b.astype(np.float32, copy=False)
    wp *= WS
    wh = wp.astype(np.float16)
    wl = (wp - wh.astype(np.float32)).astype(np.float16)
    return (
        np.ascontiguousarray(wh.reshape(128, CT * K)),
        np.ascontiguousarray(wl.reshape(128, CT * K)),
    )


def _pad_x(x: np.ndarray):
    """fp16 (hi, lo) pair of 64*x padded to CP rows (row 2208 = 64 ones)."""
    xp = np.zeros((B, CP, N), dtype=np.float32)
    xp[:, :C] = x.reshape(B, C, N)
    xp[:, C] = 1.0
    xp = xp.reshape(B, 128, CT * N)
    xs64 = xp * np.float32(XS)
    xh = xs64.astype(np.float16)
    xl = (xs64 - xh.astype(np.float32)).astype(np.float16)
    return xh, xl


def _run(x, fc_w, fc_b, flag, trace=False, trace_cores=None):
    x = np.asarray(x, dtype=np.float32)
    xh, xl = _pad_x(x)
    wh, wl = _pack_weights(np.asarray(fc_w), np.asarray(fc_b))
    in_maps = [
        {
            "xh": np.ascontiguousarray(xh[i * BPC : (i + 1) * BPC]),
            "xl": np.ascontiguousarray(xl[i * BPC : (i + 1) * BPC]),
            "wh": wh,
            "wl": wl,
        }
        for i in range(NCORES)
    ]
    nc = _get_nc()
    res = run_bass_kernel_spmd(
        nc,
        in_maps,
        core_ids=list(range(NCORES)),
        trace=trace,
        **({"trace_cores": trace_cores} if trace_cores else {}),
    )
    out = np.concatenate(
        [r["yo"].reshape(BPC, CP, N)[:, :C] for r in res.results], axis=0
    )
    out = out.reshape(B, C, H, W)
    p_r = np.concatenate([r["pr"] for r in res.results], axis=0)
    if not int(np.asarray(flag)):
        p_r = np.zeros_like(p_r)
    return (out, p_r), res


def kernel(x, fc_w, fc_b, flag):
    (out, p_r), _ = _run(x, fc_w, fc_b, flag)
    return out, p_r
